# revision 8
# baseline (speedup 1.0000x reference)
"""NetVLAD forward kernel for 8 TRN2 NeuronCores (Bass/Tile).

Reference (per batch b of 32):
  s = x @ Wk + b         (1024, 64) logits;  softmax over k -> a
  v[d,k] = sum_n a[n,k] x[n,d] + (sum_n a[n,k]) * C[d,k]
  v /= ||v||_2 over d (per k);  out = flatten(v) / ||flatten(v)||_2

Sharding: data-parallel over batch B=32 across 8 cores (4 batches/core).
Wk, b, C replicated; no collectives; host concatenates outputs.

Layout tricks (all [64, *] work is packed two-to-a-tile onto 128 partitions):
  - mm1 logits for both 512-pixel groups share one PSUM tile [128, 512]
    (group g occupies partitions 64g..64g+63), one Exp covers both.
  - mm2 vT / asum for a PAIR of batches share [128, 512] / [128, 1] PSUM.
  - softmax normalization is folded into x (x~ = x * 1/Z, per-partition
    scalar), so matmul2 operands are raw exp(e) and x~.
  - the norm tail (sqrt/recip/global-norm) is batched across all 4 batches
    at the end: ACT function switches (table loads ~1.3us each) drop to 2.
Engines: PE transposes+matmuls (bf16, fp32 PSUM accum), ACT = Exp + copies,
DVE = PSUM-touching elementwise, GPSIMD = casting DMAs + SBUF-only mults.
"""

import sys

sys.path.insert(0, "/opt/trn_rl_repo")

from contextlib import ExitStack

import numpy as np

import concourse.bacc as bacc
import concourse.bass as bass
import concourse.tile as tile
from concourse import mybir
from concourse.bass_utils import run_bass_kernel_spmd
from concourse.masks import make_identity

F32 = mybir.dt.float32
BF16 = mybir.dt.bfloat16
AX = mybir.AxisListType
OP = mybir.AluOpType
ACTF = mybir.ActivationFunctionType

B_PER_CORE = 4  # 32 batches / 8 cores
N = 1024  # H*W pixels per batch
D = 512
K = 64
EPS = 1e-12
N_CORES = 8


def build_kernel():
    nc = bacc.Bacc()
    x = nc.declare_dram_parameter("x", [B_PER_CORE * N, D], F32, isOutput=False)
    wk = nc.declare_dram_parameter("wk", [D, K], F32, isOutput=False)
    bb = nc.declare_dram_parameter("bb", [K, 1], F32, isOutput=False)
    cc = nc.declare_dram_parameter("cc", [D, K], F32, isOutput=False)
    out = nc.declare_dram_parameter("out", [B_PER_CORE, D * K], F32, isOutput=True)

    with tile.TileContext(nc) as tc, ExitStack() as ctx:
        const = ctx.enter_context(tc.tile_pool(name="const", bufs=1))
        xpool = ctx.enter_context(tc.tile_pool(name="xpool", bufs=2))
        xts = ctx.enter_context(tc.tile_pool(name="xts", bufs=3))
        sbm = ctx.enter_context(tc.tile_pool(name="sbm", bufs=2))
        nrm = ctx.enter_context(tc.tile_pool(name="nrm", bufs=2))
        # PSUM pools: 2+1+2+1+2 = 8 banks
        ps_xt = ctx.enter_context(tc.tile_pool(name="ps_xt", bufs=2, space="PSUM"))
        ps_e = ctx.enter_context(tc.tile_pool(name="ps_e", bufs=1, space="PSUM"))
        ps_s = ctx.enter_context(tc.tile_pool(name="ps_s", bufs=2, space="PSUM"))
        ps_v = ctx.enter_context(tc.tile_pool(name="ps_v", bufs=1, space="PSUM"))
        ps_m = ctx.enter_context(tc.tile_pool(name="ps_m", bufs=2, space="PSUM"))

        # ---- constants ----
        id_bf = const.tile([128, 128], BF16)
        make_identity(nc, id_bf[:])
        id_f32 = const.tile([128, 128], F32)
        make_identity(nc, id_f32[:])

        wkb = const.tile([128, 4, K], BF16)  # Wk [d, k] d-chunked, bf16
        nc.gpsimd.dma_start(out=wkb[:], in_=wk[:].rearrange("(j p) k -> p j k", p=128))
        b2_sb = const.tile([128, 1], F32)  # bias stacked for both groups
        nc.sync.dma_start(out=b2_sb[0:K, :], in_=bb[:])
        nc.sync.dma_start(out=b2_sb[K : 2 * K, :], in_=bb[:])

        ones128 = const.tile([128, 1], F32)
        nc.vector.memset(ones128[:], 1.0)
        ones_row = const.tile([1, K], F32)
        nc.vector.memset(ones_row[:], 1.0)
        eps_sb = const.tile([128, 1], F32)
        nc.vector.memset(eps_sb[:], float(EPS))

        # C^T stacked twice -> ct2 [128, 512] f32
        c_nat = const.tile([128, 4, K], F32)
        nc.sync.dma_start(out=c_nat[:], in_=cc[:].rearrange("(j p) k -> p j k", p=128))
        ct_ps = ps_m.tile([K, D], F32, tag="misc")
        for j in range(4):
            nc.tensor.transpose(
                ct_ps[:, j * 128 : (j + 1) * 128], c_nat[:, j, :], id_f32[:]
            )
        ct2 = const.tile([128, D], F32)
        nc.vector.tensor_copy(ct2[0:K, :], ct_ps[:])
        nc.vector.tensor_copy(ct2[K : 2 * K, :], ct_ps[:])

        # ---- per-batch pipeline ----
        v2 = {}
        S_all = nrm.tile([128, 2], F32, tag="sall")
        for b in range(B_PER_CORE):
            p, h = b // 2, b % 2

            xb = xpool.tile([128, 8, D], BF16, tag="xb")
            for g in range(2):
                nc.gpsimd.dma_start(
                    out=xb[:, 4 * g : 4 * (g + 1), :],
                    in_=x[b * N + 512 * g : b * N + 512 * (g + 1), :].rearrange(
                        "(i p) d -> p i d", p=128
                    ),
                )

            # -- mm1 for both groups into one PSUM tile [128, 512] --
            s_ps = ps_s.tile([128, 512], F32, tag="s")
            for g in range(2):
                for j in range(4):  # d-chunks
                    xt_ps = ps_xt.tile([128, 512], BF16, tag="xt")
                    for c in range(4):  # n-subtiles
                        nc.tensor.transpose(
                            xt_ps[:, c * 128 : (c + 1) * 128],
                            xb[:, g * 4 + c, j * 128 : (j + 1) * 128],
                            id_bf[:],
                        )
                    xt_sb = xts.tile([128, 512], BF16, tag="xt_sb")
                    nc.scalar.copy(xt_sb[:], xt_ps[:])
                    nc.tensor.matmul(
                        s_ps[K * g : K * (g + 1), :],
                        wkb[:, j, :],
                        xt_sb[:],
                        start=(j == 0),
                        stop=(j == 3),
                        skip_group_check=True,
                    )

            # -- exp(s + b) for both groups at once --
            eT = sbm.tile([128, 512], BF16, tag="eT")
            nc.scalar.activation(eT[:], s_ps[:], ACTF.Exp, bias=b2_sb[:])

            # -- transpose e back to [n, k]; Z; invZ --
            a_sb = sbm.tile([128, 8, K], BF16, tag="a")
            z_all = sbm.tile([128, 8], F32, tag="z")
            invz = sbm.tile([128, 8], F32, tag="invz")
            invz_bf = sbm.tile([128, 8], BF16, tag="invzbf")
            for g in range(2):
                e_ps = ps_e.tile([128, 4, K], BF16, tag="e")
                for c in range(4):
                    nc.tensor.transpose(
                        e_ps[:, c, :],
                        eT[K * g : K * (g + 1), c * 128 : (c + 1) * 128],
                        id_bf[K * g : K * (g + 1), K * g : K * (g + 1)],
                    )
                nc.vector.reduce_sum(z_all[:, g * 4 : (g + 1) * 4], e_ps[:], axis=AX.X)
                nc.scalar.copy(a_sb[:, 4 * g : 4 * (g + 1), :], e_ps[:])
            nc.vector.reciprocal(invz[:], z_all[:])
            nc.vector.tensor_copy(invz_bf[:], invz[:])

            # -- x~ = x * invZ (per-pixel softmax denominator folded into x) --
            xs = xpool.tile([128, 8, D], BF16, tag="xs")
            for i in range(8):
                eng = nc.vector if i % 2 == 0 else nc.gpsimd
                eng.tensor_scalar_mul(xs[:, i, :], xb[:, i, :], invz[:, i : i + 1])

            # -- mm2 + asum for the batch pair into [128, *] PSUM --
            if h == 0:
                v_ps = ps_v.tile([128, 512], F32, tag="v")
                as_ps = ps_m.tile([128, 1], F32, tag="misc")
                v2[p] = (v_ps, as_ps)
            v_ps, as_ps = v2[p]
            for i in range(8):
                nc.tensor.matmul(
                    v_ps[K * h : K * (h + 1), :],
                    a_sb[:, i, :],
                    xs[:, i, :],
                    start=(i == 0),
                    stop=(i == 7),
                    skip_group_check=True,
                )
            for i in range(8):
                nc.tensor.matmul(
                    as_ps[K * h : K * (h + 1), :],
                    a_sb[:, i, :],
                    invz_bf[:, i : i + 1],
                    start=(i == 0),
                    stop=(i == 7),
                    skip_group_check=True,
                )

            # -- pair complete: v = vT + asum*C^T, S_k = sum_d v^2 --
            if h == 1:
                asum = nrm.tile([128, 1], F32, tag="asum")
                nc.vector.tensor_copy(asum[:], as_ps[:])
                vc = nrm.tile([128, D], F32, tag="vc")
                nc.gpsimd.tensor_scalar_mul(vc[:], ct2[:], asum[:])
                vv = nrm.tile([128, D], F32, tag="vv")
                nc.vector.tensor_add(vv[:], vc[:], v_ps[:])
                v2[p] = vv
                sq = nrm.tile([128, D], F32, tag="sq")
                nc.gpsimd.tensor_mul(sq[:], vv[:], vv[:])
                nc.vector.reduce_sum(S_all[:, p : p + 1], sq[:], axis=AX.X)

        # ---- norm tail (all 4 batches) ----
        q_all = nrm.tile([128, 2], F32, tag="qall")
        nc.scalar.activation(q_all[:], S_all[:], ACTF.Sqrt, bias=eps_sb[:])
        rsq = nrm.tile([128, 2], F32, tag="rsq")
        nc.vector.reciprocal(rsq[:], q_all[:])
        # t = S * rsq^2 = S/(S+eps)
        t_t = nrm.tile([128, 2], F32, tag="tt")
        nc.vector.tensor_mul(t_t[:], rsq[:], rsq[:])
        nc.vector.tensor_mul(t_t[:], t_t[:], S_all[:])
        # gss[b] = sum_k t[k, b] ; g = 1/sqrt(gss+eps); gb = g broadcast [128, 2]
        gss_ps = ps_m.tile([1, 4], F32, tag="misc")
        for b in range(4):
            p, h = b // 2, b % 2
            nc.tensor.matmul(
                gss_ps[:, b : b + 1],
                t_t[K * h : K * (h + 1), p : p + 1],
                ones128[K * h : K * (h + 1), :],
                start=True,
                stop=True,
                skip_group_check=True,
            )
        q4 = nrm.tile([1, 4], F32, tag="q4")
        nc.scalar.activation(q4[:], gss_ps[:], ACTF.Sqrt, bias=eps_sb[:1, :])
        ginv = nrm.tile([1, 4], F32, tag="ginv")
        nc.vector.reciprocal(ginv[:], q4[:])
        gb_ps = ps_m.tile([128, 2], F32, tag="misc")
        for b in range(4):
            p, h = b // 2, b % 2
            nc.tensor.matmul(
                gb_ps[K * h : K * (h + 1), p : p + 1],
                ones_row[:],
                ginv[:, b : b + 1],
                start=True,
                stop=True,
                skip_group_check=True,
            )
        sc2 = nrm.tile([128, 2], F32, tag="sc2")
        nc.vector.tensor_mul(sc2[:], rsq[:], gb_ps[:])

        # ---- scale, transpose back to [d, k], store ----
        for p in range(2):
            vf = nrm.tile([128, D], F32, tag="vf")
            nc.vector.tensor_scalar_mul(vf[:], v2[p][:], sc2[:, p : p + 1])
            for h in range(2):
                b = 2 * p + h
                o_ps = ps_m.tile([128, 4, K], F32, tag="misc")
                for j in range(4):
                    nc.tensor.transpose(
                        o_ps[:, j, :],
                        vf[K * h : K * (h + 1), j * 128 : (j + 1) * 128],
                        id_f32[K * h : K * (h + 1), K * h : K * (h + 1)],
                    )
                o_sb = nrm.tile([128, 4, K], F32, tag="osb")
                nc.scalar.copy(o_sb[:], o_ps[:])
                nc.sync.dma_start(
                    out=out[b].rearrange("(j p k) -> p j k", j=4, p=128, k=K),
                    in_=o_sb[:],
                )

    nc.compile()
    return nc


_CACHED_NC = None


def _get_nc():
    global _CACHED_NC
    if _CACHED_NC is None:
        _CACHED_NC = build_kernel()
    return _CACHED_NC


def kernel(x, Wk, b, C):
    """Full-input NetVLAD forward. x (32,32,32,512) f32 -> out (32, 32768) f32."""
    B = x.shape[0]
    x2 = np.ascontiguousarray(x, dtype=np.float32).reshape(B, N, D)
    bpc = B // N_CORES
    in_maps = []
    for c in range(N_CORES):
        in_maps.append(
            {
                "x": x2[c * bpc : (c + 1) * bpc].reshape(bpc * N, D),
                "wk": np.ascontiguousarray(Wk, dtype=np.float32),
                "bb": np.ascontiguousarray(b, dtype=np.float32).reshape(K, 1),
                "cc": np.ascontiguousarray(C, dtype=np.float32),
            }
        )
    nc = _get_nc()
    res = run_bass_kernel_spmd(nc, in_maps, list(range(N_CORES)))
    return np.concatenate([res.results[c]["out"] for c in range(N_CORES)], axis=0)
